# revision 20
# baseline (speedup 1.0000x reference)
"""Bass/Tile kernel for nn_Attn_40424232189956 on 8 trn2 NeuronCores.

GQA attention block: q/k/v proj + rmsnorm + rope + causal attention + out proj.
B=2, T=2048, D=2048, NH=16, NKV=4, HD=128.

Sharding: tensor-parallel over heads. Each core owns 2 q-heads + the 1 kv-head
they read (q heads 2c,2c+1 -> kv head c//2), computes a full [B*T, D] partial
of the output projection; host sums the 8 partials.

v2 layout notes (cost model: engine time ~ free-dim cols; DMA is a serial
resource at ~360 B/ns with 625ns issue overhead per transfer):
- All streamed data bf16: x, weights (kw+vw merged for >=512B rows), rope
  tables, q/k/v sbuf tiles, attention probabilities, output partials.
- One DMA per x chunk ([128, 16, 512]); outputs assembled to [128, 2048]
  before a single store. Weight DMAs ordered so matmuls start at ~4us.
- norm+rope: Square -> ones-matmul -> Rsqrt(bias) -> one psum*rfac mul, then
  2 table muls + add + sub in bf16 (DVE 2x mode), using [cos|sin] and
  [sin|cos] stacked tables; no gpsimd, no separate reciprocal.
- Attention with transposed scores sT[k, q]; causal handled by shrinking the
  moving window to [qoff:512] on diagonal k-blocks plus ONE [128,128]
  mask-add per diagonal block. Softmax denominator via ones-column matmul
  accumulated in one shared psum bank (rows 0/1 = the 2 heads).
- j-loop software-pipelined by one slot (stile pair of j+1 issued before
  sms/yts of j); stile tiles alternate between two psum pools so 4 banks
  back the pipeline; oproj of group g-1 is interposed into group g to hide
  the denominator/broadcast chain.
"""

import numpy as np
import ml_dtypes

B, T, D = 2, 2048, 2048
NH, NKV = 16, 4
HD = 128
BT = B * T            # 4096
NCORES = 8
HPC = 2               # q heads per core
NKT = D // 128        # 16 contraction tiles for projections
CHUNK = 512
EPS = float(np.finfo(np.float32).eps)
MASK_NEG = -30000.0
BF16 = ml_dtypes.bfloat16


def _rope_tables():
    # Matches reference.rotary_tables for T=2048 > tsl=1024 (NTK branch).
    hd = np.float32(HD)
    ar = (np.arange(0, HD, 2, dtype=np.float32) / hd).astype(np.float32)
    expo = np.power(np.float32(HD / (HD - 2.0)), ar, dtype=np.float32)
    inv = (np.float32(1.0)
           / (np.float32(10000.0)
              * np.power(np.float32(T / 1024.0), expo, dtype=np.float32)))
    f = np.outer(np.arange(T, dtype=np.float32), inv.astype(np.float32))
    return (np.cos(f).astype(np.float32).T.copy(),
            np.sin(f).astype(np.float32).T.copy())  # [64, T] hd-major


def _build_program():
    import concourse.bass as bass
    import concourse.mybir as mybir
    import concourse.tile as tile
    from concourse import bacc
    from concourse.masks import make_identity

    f32 = mybir.dt.float32
    f32r = mybir.dt.float32r
    bf16 = mybir.dt.bfloat16
    nc = bacc.Bacc("TRN2", target_bir_lowering=False)

    xT = nc.dram_tensor("xT", [D, BT], bf16, kind="ExternalInput")
    qwT = nc.dram_tensor("qwT", [D, HPC * HD], bf16, kind="ExternalInput")
    kvwT = nc.dram_tensor("kvwT", [D, 2 * HD], bf16, kind="ExternalInput")
    owT = nc.dram_tensor("owT", [HPC * HD, D], bf16, kind="ExternalInput")
    csd = nc.dram_tensor("csd", [128, T], bf16, kind="ExternalInput")
    csd2 = nc.dram_tensor("csd2", [128, T], bf16, kind="ExternalInput")
    maskd = nc.dram_tensor("maskd", [128, 128], f32, kind="ExternalInput")
    normod = nc.dram_tensor("normod", [128, 3, 128], f32r, kind="ExternalInput")
    normbd = nc.dram_tensor("normbd", [128, 3], f32, kind="ExternalInput")
    outd = nc.dram_tensor("o", [BT, D], bf16, kind="ExternalOutput")

    sq_ = mybir.ActivationFunctionType.Square
    sqrt_ = mybir.ActivationFunctionType.Sqrt
    exp_ = mybir.ActivationFunctionType.Exp

    with tile.TileContext(nc) as tc:
        with (
            tc.tile_pool(name="wpool", bufs=1) as wpool,
            tc.tile_pool(name="xpool", bufs=2) as xpool,
            tc.tile_pool(name="big", bufs=2) as big,
            tc.tile_pool(name="ybp", bufs=2) as ybp,
            tc.tile_pool(name="ntmp", bufs=2) as ntmp,
            tc.tile_pool(name="rtmp", bufs=3) as rtmp,
            tc.tile_pool(name="atmp", bufs=2) as atmp,
            tc.tile_pool(name="rbp", bufs=4) as rbp,
            tc.tile_pool(name="ppool", bufs=12) as ppool,
            tc.tile_pool(name="opool", bufs=2) as opool,
            # PSUM: 8 banks total.
            tc.tile_pool(name="pp", bufs=2, space="PSUM") as pp,
            tc.tile_pool(name="pa", bufs=3, space="PSUM") as pa,
            tc.tile_pool(name="py", bufs=2, space="PSUM") as py,
            tc.tile_pool(name="psv", bufs=1, space="PSUM") as psv,
        ):
            # ---- resident weights / tables (issue order = DMA order) ----
            qw_s = wpool.tile([128, NKT, HPC * HD], bf16)
            qwT_re = qwT.rearrange("(ko p) m -> p ko m", p=128)
            for kq in range(4):
                nc.sync.dma_start(qw_s[:, 4 * kq:4 * (kq + 1), :],
                                  qwT_re[:, 4 * kq:4 * (kq + 1), :])
            # deferred weight loads (emitted inside first chunk, see below)
            kvw_s = wpool.tile([128, NKT, 2 * HD], bf16)
            normo_s = wpool.tile([128, 3, 128], f32r)
            normb_s = wpool.tile([128, 3], f32)
            cs_s = wpool.tile([128, T], bf16)   # rows 0:64 cos, 64:128 sin
            cs2_s = wpool.tile([128, T], bf16)  # rows 0:64 sin, 64:128 cos
            ow_s = wpool.tile([128, HPC, D], bf16)
            mask_s = wpool.tile([128, 128], f32)

            ones_bf = wpool.tile([128, 1], bf16)
            nc.vector.memset(ones_bf[:], 1.0)
            ident = wpool.tile([128, 128], bf16)
            make_identity(nc, ident[:])

            xT_re = xT.rearrange("(ko p) m -> p ko m", p=128)

            def norm_rope(pt, ni, dst, pos0):
                """pt: psum [128 feat, 512 tok] f32; ni: 0/1 q-head, 2 k;
                dst: bf16 sbuf [128, 512] slice; pos0: seq position.
                All DVE ops keep input partition windows aligned; only the
                output window shifts (the pattern v1 validated)."""
                sq = ntmp.tile([128, CHUNK], f32r, tag="sq")
                nc.scalar.activation(out=sq[:], in_=pt[:], func=sq_)
                nb = psv.tile([128, CHUNK], f32, tag="aux", name=f"nb{ni}")
                nc.tensor.matmul(nb[:], normo_s[:, ni, :], sq[:],
                                 start=True, stop=True)
                rs = ntmp.tile([128, CHUNK], f32, tag="rs")
                nc.scalar.activation(out=rs[:], in_=nb[:], func=sqrt_,
                                     bias=normb_s[:, ni:ni + 1], scale=1.0)
                rfac = ntmp.tile([128, CHUNK], f32, tag="rfac")
                nc.vector.reciprocal(rfac[:], rs[:])
                qn = rtmp.tile([128, CHUNK], bf16, tag="qn")
                nc.vector.tensor_mul(qn[:], pt[:], rfac[:])
                csl = cs_s[0:64, pos0:pos0 + CHUNK]       # cos @ base 0
                snh = cs_s[64:128, pos0:pos0 + CHUNK]     # sin @ base 64
                snl = cs2_s[0:64, pos0:pos0 + CHUNK]      # sin @ base 0
                csh = cs2_s[64:128, pos0:pos0 + CHUNK]    # cos @ base 64
                p1 = rtmp.tile([64, CHUNK], bf16, tag="p1")
                p2 = rtmp.tile([64, CHUNK], bf16, tag="p2")
                p3 = rtmp.tile([64, CHUNK], bf16, tag="p3")
                p4 = rtmp.tile([64, CHUNK], bf16, tag="p4")
                nc.vector.tensor_mul(p1[:], qn[0:64, :], csl)
                nc.vector.tensor_mul(p2[:], qn[64:128, :], snh)
                nc.vector.tensor_add(dst[0:64, :], p1[:], p2[:])
                nc.vector.tensor_mul(p3[:], qn[64:128, :], csh)
                nc.vector.tensor_mul(p4[:], qn[0:64, :], snl)
                nc.vector.tensor_sub(dst[64:128, :], p3[:], p4[:])

            tiles = {}
            # deferred-work FIFO: each entry emitted as the `mid` hook of a
            # later matmul pass so norm chains overlap the next pass's mms
            pending_mid = []

            def flush_mid():
                while pending_mid:
                    pending_mid.pop(0)()

            def norm_a(pt, ni):
                """sq part of the norm chain (emit right after pass stop)."""
                sq = ntmp.tile([128, CHUNK], f32r, tag="sq")
                nc.scalar.activation(out=sq[:], in_=pt[:], func=sq_)
                return sq

            def norm_b(pt, sq, ni, dst, pos0):
                """nb matmul + rsqrt + rope (the `mid` hook body)."""
                nb = psv.tile([128, CHUNK], f32, tag="aux", name=f"nb{ni}")
                nc.tensor.matmul(nb[:], normo_s[:, ni, :], sq[:],
                                 start=True, stop=True)
                rs = ntmp.tile([128, CHUNK], f32, tag="rs")
                nc.scalar.activation(out=rs[:], in_=nb[:], func=sqrt_,
                                     bias=normb_s[:, ni:ni + 1], scale=1.0)
                rfac = ntmp.tile([128, CHUNK], f32, tag="rfac")
                nc.vector.reciprocal(rfac[:], rs[:])
                qn = rtmp.tile([128, CHUNK], bf16, tag="qn")
                nc.vector.tensor_mul(qn[:], pt[:], rfac[:])
                csl = cs_s[0:64, pos0:pos0 + CHUNK]       # cos @ base 0
                snh = cs_s[64:128, pos0:pos0 + CHUNK]     # sin @ base 64
                snl = cs2_s[0:64, pos0:pos0 + CHUNK]      # sin @ base 0
                csh = cs2_s[64:128, pos0:pos0 + CHUNK]    # cos @ base 64
                p1 = rtmp.tile([64, CHUNK], bf16, tag="p1")
                p2 = rtmp.tile([64, CHUNK], bf16, tag="p2")
                p3 = rtmp.tile([64, CHUNK], bf16, tag="p3")
                p4 = rtmp.tile([64, CHUNK], bf16, tag="p4")
                nc.vector.tensor_mul(p1[:], qn[0:64, :], csl)
                nc.vector.tensor_mul(p2[:], qn[64:128, :], snh)
                nc.vector.tensor_add(dst[0:64, :], p1[:], p2[:])
                nc.vector.tensor_mul(p3[:], qn[64:128, :], csh)
                nc.vector.tensor_mul(p4[:], qn[0:64, :], snl)
                nc.vector.tensor_sub(dst[64:128, :], p3[:], p4[:])

            def v_finalize(b, ci, pv, vtok, pos0):
                # v: psum [hd, tok] -> bf16 sbuf, PE-transpose to token-major
                vtmp = atmp.tile([128, CHUNK], bf16, tag="vtmp",
                                 name=f"vtmp_{b}_{ci}")
                nc.scalar.copy(out=vtmp[:], in_=pv[:])
                vps = psv.tile([128, CHUNK], bf16, tag="aux",
                               name=f"vps_{b}_{ci}")
                for tb in range(4):
                    nc.tensor.transpose(
                        vps[:, tb * 128:(tb + 1) * 128],
                        vtmp[:, tb * 128:(tb + 1) * 128], ident[:])
                nc.scalar.copy(out=vtok[:, pos0:pos0 + CHUNK], in_=vps[:])

            def proj_pass(pt, w_s, fsel, xc):
                """16 accumulating matmuls; runs one pending mid at ko==4."""
                for ko in range(NKT):
                    if ko == 4 and pending_mid:
                        pending_mid.pop(0)()
                    nc.tensor.matmul(pt[:], w_s[:, ko, fsel], xc[:, ko, :],
                                     start=(ko == 0), stop=(ko == NKT - 1))

            def proj_chunk(b, ci):
                if ci == 0:
                    tiles[b] = (
                        big.tile([128, HPC, T], bf16, tag="qT", name=f"qT{b}"),
                        big.tile([128, T], bf16, tag="kT", name=f"kT{b}"),
                        big.tile([128, T], bf16, tag="vtok", name=f"vtok{b}"),
                    )
                qT, kT, vtok = tiles[b]
                pos0 = ci * CHUNK
                t0 = b * T + pos0
                xc = xpool.tile([128, NKT, CHUNK], bf16, tag="xc",
                                name=f"xc_{b}_{ci}")
                if b == 0 and ci == 0:
                    # split first chunk's load so matmuls start early
                    for kq in range(4):
                        nc.sync.dma_start(
                            xc[:, 4 * kq:4 * (kq + 1), :],
                            xT_re[:, 4 * kq:4 * (kq + 1), t0:t0 + CHUNK])
                    # deferred resident loads, after the critical x tiles
                    nc.sync.dma_start(normo_s[:], normod[:])
                    nc.sync.dma_start(normb_s[:], normbd[:])
                    nc.sync.dma_start(kvw_s[:], kvwT.rearrange(
                        "(ko p) m -> p ko m", p=128))
                else:
                    nc.sync.dma_start(xc[:], xT_re[:, :, t0:t0 + CHUNK])
                if b == 0:
                    # rope tables stream per-chunk, just ahead of their use
                    nc.sync.dma_start(cs_s[:, pos0:pos0 + CHUNK],
                                      csd[:, pos0:pos0 + CHUNK])
                    nc.sync.dma_start(cs2_s[:, pos0:pos0 + CHUNK],
                                      csd2[:, pos0:pos0 + CHUNK])
                if b == 0 and ci == 1:
                    nc.sync.dma_start(ow_s[:], owT.rearrange(
                        "(h p) n -> p h n", p=128))
                    nc.sync.dma_start(mask_s[:], maskd[:])
                dsts = [qT[:, 0, pos0:pos0 + CHUNK], qT[:, 1, pos0:pos0 + CHUNK],
                        kT[:, pos0:pos0 + CHUNK]]
                fsels = [slice(0, 128), slice(128, 256),
                         slice(0, 128), slice(128, 256)]
                wsels = [qw_s, qw_s, kvw_s, kvw_s]
                for fi in range(4):
                    pt = pp.tile([128, CHUNK], f32, tag="pb",
                                 name=f"pt_{b}_{ci}_{fi}")
                    proj_pass(pt, wsels[fi], fsels[fi], xc)
                    if fi < 3:
                        sq = norm_a(pt, fi)
                        pending_mid.append(
                            lambda pt=pt, sq=sq, fi=fi, dst=dsts[fi]:
                            norm_b(pt, sq, fi, dst, pos0))
                    else:
                        pending_mid.append(
                            lambda pv=pt: v_finalize(b, ci, pv, vtok, pos0))

            # ---- attention ----
            attn_state = {}

            def emit_stile_exp(b, g, j, h, qT, kT):
                """score tile + mask + exp for (j, h); returns pj + window."""
                q0 = g * 512
                r = j - 4 * g
                qoff = 128 * r if r > 0 else 0
                k0 = j * 128
                pool = pa if h == 0 else pp
                stile = pool.tile([128, 512], f32, tag="pb",
                                  name=f"st_{b}_{g}_{j}_{h}")
                nc.tensor.matmul(stile[:, qoff:512], kT[:, k0:k0 + 128],
                                 qT[:, h, q0 + qoff:q0 + 512],
                                 start=True, stop=True)
                if 0 <= r < 4:
                    nc.vector.tensor_add(stile[:, qoff:qoff + 128],
                                         stile[:, qoff:qoff + 128], mask_s[:])
                pj = ppool.tile([128, 512], bf16, tag="pj",
                                name=f"pj_{b}_{g}_{j}_{h}")
                nc.scalar.activation(out=pj[:, qoff:512], in_=stile[:, qoff:512],
                                     func=exp_)
                return pj, qoff

            def emit_acc(b, g, j, h, pj, qoff, smb, yts, vtok, kg):
                k0 = j * 128
                st = (j == 0)
                sp = (j == kg - 1)
                nc.tensor.matmul(smb[64 * h:64 * h + 1, qoff:512], ones_bf[:],
                                 pj[:, qoff:512], start=st, stop=sp,
                                 skip_group_check=True)
                nc.tensor.matmul(yts[h][:, qoff:512], vtok[:, k0:k0 + 128],
                                 pj[:, qoff:512], start=st, stop=sp,
                                 skip_group_check=True)

            def emit_denom_ybg(b, g):
                """yts psum -> bf16 sbuf (frees banks fast), then
                rr -> broadcast -> ybg entirely off PSUM."""
                smb, yts = attn_state[(b, g)]["smb"], attn_state[(b, g)]["yts"]
                ybg = ybp.tile([128, HPC, 512], bf16, tag="ybg",
                               name=f"ybg_{b}_{g}")
                ysb = [atmp.tile([128, 512], bf16, tag=f"ysb{h}",
                                 name=f"ysb_{b}_{g}_{h}") for h in range(HPC)]
                for h in range(HPC):
                    nc.scalar.copy(out=ysb[h][:], in_=yts[h][:])
                for h in range(HPC):
                    rr = atmp.tile([1, 512], f32, tag="rr",
                                   name=f"rr_{b}_{g}_{h}")
                    nc.vector.reciprocal(rr[:], smb[64 * h:64 * h + 1, :])
                    rb = rbp.tile([128, 512], f32, tag="rb",
                                  name=f"rb_{b}_{g}_{h}")
                    nc.gpsimd.partition_broadcast(rb[:], rr[:])
                    nc.vector.tensor_mul(ybg[:, h, :], ysb[h][:], rb[:])
                attn_state[(b, g)]["ybg"] = ybg

            def emit_oproj(b, g):
                ybg = attn_state[(b, g)]["ybg"]
                q0 = g * 512
                for tb in range(4):
                    row0 = b * T + q0 + tb * 128
                    obuf = opool.tile([128, D], bf16, tag="obuf",
                                      name=f"ob_{b}_{g}_{tb}")
                    for oc in range(4):
                        ops = pa.tile([128, 512], f32, tag="pb",
                                      name=f"op_{b}_{g}_{tb}_{oc}")
                        nc.tensor.matmul(ops[:],
                                         ybg[:, 0, tb * 128:(tb + 1) * 128],
                                         ow_s[:, 0, oc * 512:(oc + 1) * 512],
                                         start=True, stop=False)
                        nc.tensor.matmul(ops[:],
                                         ybg[:, 1, tb * 128:(tb + 1) * 128],
                                         ow_s[:, 1, oc * 512:(oc + 1) * 512],
                                         start=False, stop=True)
                        if oc % 2 == 0:
                            nc.vector.tensor_copy(
                                out=obuf[:, oc * 512:(oc + 1) * 512], in_=ops[:])
                        else:
                            nc.scalar.copy(
                                out=obuf[:, oc * 512:(oc + 1) * 512], in_=ops[:])
                    nc.sync.dma_start(outd[row0:row0 + 128, :], obuf[:])

            def attn_batch(b, pending_oproj):
                """Flat slot stream over all 4 groups: stiles of slot s,
                then accs of slot s-1 (covers exp latency even across group
                boundaries). pending_oproj interposed a few slots in."""
                qT, kT, vtok = tiles[b]
                slots = [(g, j) for g in range(4) for j in range(4 * (g + 1))]
                n = len(slots)
                prev = None
                for s in range(n + 1):
                    if s < n:
                        g, j = slots[s]
                        cur = (g, j, [emit_stile_exp(b, g, j, h, qT, kT)
                                      for h in range(HPC)])
                    if s == 3 and pending_oproj is not None:
                        emit_oproj(*pending_oproj)
                    # oproj of each finished group, 2 slots after its denom
                    if s == 6:
                        emit_oproj(b, 0)
                    elif s == 14:
                        emit_oproj(b, 1)
                    elif s == 26:
                        emit_oproj(b, 2)
                    if prev is not None:
                        gp, jp, pjs = prev
                        kg = 4 * (gp + 1)
                        if jp == 0:
                            yts = [py.tile([128, 512], f32, tag="pb",
                                           name=f"yt_{b}_{gp}_{h}")
                                   for h in range(HPC)]
                            smb = psv.tile([128, CHUNK], f32, tag="aux",
                                           name=f"smb_{b}_{gp}")
                            attn_state[(b, gp)] = {"smb": smb, "yts": yts}
                        st = attn_state[(b, gp)]
                        for h in range(HPC):
                            pj, qoff = pjs[h]
                            emit_acc(b, gp, jp, h, pj, qoff, st["smb"],
                                     st["yts"], vtok, kg)
                        if jp == kg - 1:
                            emit_denom_ybg(b, gp)
                    prev = cur

            pending = None
            for b in range(B):
                for ci in range(4):
                    proj_chunk(b, ci)
                    if b == 1 and ci == 0 and pending is not None:
                        emit_oproj(*pending)
                        pending = None
                flush_mid()
                # preload the exp activation table while proj tail drains
                dummy = atmp.tile([128, 1], bf16, tag="dummy",
                                  name=f"dummy_{b}")
                nc.scalar.activation(out=dummy[:], in_=ones_bf[:], func=exp_)
                attn_batch(b, pending)
                pending = (b, 3)
            emit_oproj(*pending)

    nc.compile()
    return nc


_CACHED = {}
LAST_EXEC_NS = None


def _run(nc, in_maps, **kwargs):
    from concourse.bass_utils import run_bass_kernel_spmd
    return run_bass_kernel_spmd(nc, in_maps, core_ids=list(range(NCORES)),
                                **kwargs)


def _make_in_maps(x, qw, kw, vw, ow, qg):
    xTf = np.ascontiguousarray(x.reshape(BT, D).T).astype(BF16)  # [D, BT]
    cosT, sinT = _rope_tables()
    cossin = np.concatenate([cosT, sinT], axis=0).astype(BF16)  # [128,T]
    sincos = np.concatenate([sinT, cosT], axis=0).astype(BF16)

    ktl = np.arange(128, dtype=np.int64)[:, None]
    qtl = np.arange(128, dtype=np.int64)[None, :]
    mask = np.where(qtl >= ktl, 0.0, MASK_NEG).astype(np.float32)  # [128,128]

    in_maps = []
    for c in range(NCORES):
        h0 = HPC * c
        kvh = (h0 * NKV) // NH  # == c // 2
        qwT_c = np.ascontiguousarray(
            qw[h0 * HD:(h0 + HPC) * HD, :].T).astype(BF16)
        kvwT_c = np.ascontiguousarray(np.concatenate(
            [kw[kvh * HD:(kvh + 1) * HD, :].T,
             vw[kvh * HD:(kvh + 1) * HD, :].T], axis=1)).astype(BF16)
        owT_c = np.ascontiguousarray(
            ow[:, h0 * HD:(h0 + HPC) * HD].T).astype(BF16)
        # norm constants: s_i folds qg gain and 1/sqrt(HD) attention scale
        s = np.array([qg[h0] / np.sqrt(HD), qg[h0 + 1] / np.sqrt(HD), 1.0],
                     np.float32)
        normo = np.broadcast_to(
            (1.0 / (HD * s * s))[None, :, None], (128, 3, 128)
        ).astype(np.float32).copy()
        normb = np.broadcast_to(
            (EPS / (s * s))[None, :], (128, 3)).astype(np.float32).copy()
        in_maps.append({
            "xT": xTf, "qwT": qwT_c, "kvwT": kvwT_c, "owT": owT_c,
            "csd": cossin, "csd2": sincos, "maskd": mask,
            "normod": normo, "normbd": normb,
        })
    return in_maps


def kernel(x, qw, kw, vw, ow, qg):
    global LAST_EXEC_NS
    x = np.ascontiguousarray(x, dtype=np.float32)
    qw = np.asarray(qw, dtype=np.float32)
    kw = np.asarray(kw, dtype=np.float32)
    vw = np.asarray(vw, dtype=np.float32)
    ow = np.asarray(ow, dtype=np.float32)
    qg = np.asarray(qg, dtype=np.float32)

    if "nc" not in _CACHED:
        _CACHED["nc"] = _build_program()
    nc = _CACHED["nc"]

    in_maps = _make_in_maps(x, qw, kw, vw, ow, qg)
    res = _run(nc, in_maps)
    LAST_EXEC_NS = res.exec_time_ns
    out = res.results[0]["o"].astype(np.float64)
    for c in range(1, NCORES):
        out += res.results[c]["o"].astype(np.float64)
    return out.astype(np.float32).reshape(B, T, D)
